# revision 19
# baseline (speedup 1.0000x reference)
"""Trainium2 Bass kernel for Enformer-style relative-position attention.

Problem: b=2, n=1536, dim=1536, 8 heads, dk=64, dv=192, rel-pos features=192.

Sharding: pure sequence sharding, no collectives. 8 cores = 2 batches x 4
query-row slices of 384. Each core computes full k/v for its batch
(duplicated within the 4-core batch group), attention + output projection
for its 384 query rows. Host concatenates the 8 (384, 1536) slices.

All matmuls run in bf16. A single bf16 x^T resident serves the k, q and
v projections. The rel-pos band rel_k^T is computed once per head-pair
over a 1919-wide window covering all three query tiles.

relative_shift is realized as a DRAM round trip: the pre-shift band
S_pre (128 x 1663) is written row-major to a flat DRAM scratch, and read
back with row stride 1662 starting at offset 127, which lands
shifted[p, j] = S_pre[p, 127 - p + j] exactly.

The shifted band is accumulated onto the content logits directly in PSUM
via an identity matmul, and the softmax exp reads PSUM. The attention
stages are emitted in a software-pipelined order (shift runs LAG=3 heads
ahead, attnv runs LAG2=2 heads behind) because the Tile scheduler emits
a static per-engine order.
"""

import math
import os

import ml_dtypes
import numpy as np

import concourse.bass as bass
import concourse.mybir as mybir
import concourse.tile as tile
from concourse import bacc
from concourse.bass_utils import run_bass_kernel_spmd
from concourse.masks import make_identity

BF16 = ml_dtypes.bfloat16
F32 = mybir.dt.float32
BF = mybir.dt.bfloat16
AF = mybir.ActivationFunctionType
AX = mybir.AxisListType
ALU = mybir.AluOpType

P = 128
N = 1536          # sequence length
D = 1536          # model dim
H = 8             # heads
DK = 64           # key dim per head
DV = 192          # value dim per head
HDV = H * DV      # 1536
NB = 384          # query rows per core
NT = NB // P      # q-tiles per core = 3
KC = D // P       # contraction chunks over model dim = 12
WB = N + P - 1    # pre-shift band width = 1663
RBS = WB - 1      # readback row stride = 1662
FLAT = P * WB     # flat scratch elements per (tile, head) = 212864
NRF = 192         # rel-pos feature size
BW = WB + 2 * P   # full band window per head-pair = 1919
BS = BW + 1       # band column stride = 1920

LAG = 3           # shift round-trip emitted this many heads ahead
LAG2 = 3          # attnv emitted this many heads behind
DV1 = DV + 1      # per-head v block incl. ones column for the softmax sum
VW = H * DV1      # v chunk width = 1544


def _np_positions():
    """numpy replication of reference.get_positional_embed(1536, 192)."""
    from scipy.special import gammaln as sp_gammaln

    n, feature_size = N, NRF
    dist = np.arange(-n + 1, n)
    adist = np.abs(dist).astype(np.float64)[:, None]
    num_basis = feature_size // 6
    max_range = math.log(n) / math.log(2.0)
    half_life = 2.0 ** np.linspace(3.0, max_range, num_basis)
    feat_exp = np.exp(-math.log(2.0) / half_life[None, :] * adist)
    center_widths = 2.0 ** np.arange(1, num_basis + 1) - 1.0
    feat_cm = (center_widths[None, :] > adist).astype(np.float64)
    stddev = n / (2 * num_basis)
    start_mean = n / num_basis
    mean = np.linspace(start_mean, float(n), num_basis)[None, :]
    concentration = (mean / stddev) ** 2
    rate = mean / (stddev**2)
    with np.errstate(divide="ignore", invalid="ignore"):
        xl = np.where(
            adist > 0,
            (concentration - 1.0) * np.log(np.where(adist > 0, adist, 1.0)),
            0.0,
        )
        xl = np.where((adist == 0) & (concentration - 1.0 != 0), -np.inf, xl)
    log_unnorm = xl - rate * adist
    log_norm = sp_gammaln(concentration) - concentration * np.log(rate)
    prob = np.exp(log_unnorm - log_norm) + 1e-8
    feat_gamma = prob / np.max(prob, axis=-1, keepdims=True)
    emb = np.concatenate([feat_exp, feat_cm, feat_gamma], axis=-1)
    sign = np.sign(dist).astype(np.float64)[:, None]
    return np.concatenate([emb, sign * emb], axis=-1).astype(np.float32)


def _build_nc():
    nc = bacc.Bacc("TRN2", target_bir_lowering=False)

    xT = nc.dram_tensor("xT", [D, N], BF, kind="ExternalInput")
    xqT = nc.dram_tensor("xqT", [D, NB], BF, kind="ExternalInput")
    wqT = nc.dram_tensor("wqT", [D, H * DK], BF, kind="ExternalInput")
    wkT = nc.dram_tensor("wkT", [D, H * DK], BF, kind="ExternalInput")
    wvT = nc.dram_tensor("wvT", [D, HDV], BF, kind="ExternalInput")
    woT = nc.dram_tensor("woT", [HDV, D], BF, kind="ExternalInput")
    wrTa = nc.dram_tensor("wrTa", [P, H * DK], BF, kind="ExternalInput")
    wrTb = nc.dram_tensor("wrTb", [NRF - P, H * DK], BF, kind="ExternalInput")
    # positions^T band window (covers all 3 local q-tiles), feature-split
    pba = nc.dram_tensor("pba", [P, BW], BF, kind="ExternalInput")
    pbb = nc.dram_tensor("pbb", [NRF - P, BW], BF, kind="ExternalInput")
    rcb = nc.dram_tensor("rcb", [P, 4], F32, kind="ExternalInput")
    rpb = nc.dram_tensor("rpb", [P, 4], F32, kind="ExternalInput")
    bo = nc.dram_tensor("bo", [1, D], BF, kind="ExternalInput")

    out = nc.dram_tensor("out", [NB, D], F32, kind="ExternalOutput")

    scale = DK ** -0.5
    iters = [(t, h) for t in range(NT) for h in range(H)]
    rbts = {}     # iter idx -> readback tile
    attnvs = {}   # tile idx -> attnv tile

    with tile.TileContext(nc) as tc:
        with (
            tc.tile_pool(name="res", bufs=1) as res,
            tc.tile_pool(name="psum_ps", bufs=3, space="PSUM") as prot,
            tc.tile_pool(name="psum_pp", bufs=2, space="PSUM") as ppp,
            tc.tile_pool(name="stg", bufs=2) as stgp,
            tc.tile_pool(name="rb", bufs=4) as rbp,
            tc.tile_pool(name="sm", bufs=8) as smp,
            tc.tile_pool(name="dram", bufs=8, space="DRAM") as dsc,
        ):
            # long-lived SBUF residents
            kT_sb = res.tile([P, 4 * N], BF, tag="kT")            # 12 KB/part
            v_sb = res.tile([P, KC * VW], BF, tag="v")            # 37 KB/part
            qcT_sb = res.tile([P, 4 * NB], BF, tag="qcT")
            qpT_sb = res.tile([P, 4 * NB], BF, tag="qpT")
            band_sb = res.tile([P, 4 * BS], BF, tag="band")       # 15.4 KB/part
            rcb_sb = res.tile([P, 4], F32, tag="rcb")
            rpb_sb = res.tile([P, 4], F32, tag="rpb")
            bo_sb = res.tile([1, D], BF, tag="bo")
            ones_sb = res.tile([1, P], BF, tag="ones")
            ident_sb = res.tile([P, P], BF, tag="ident")
            wra_sb = res.tile([P, H * DK], BF, tag="wra")
            wrb_sb = res.tile([NRF - P, H * DK], BF, tag="wrb")
            pa_sb = res.tile([P, BW], BF, tag="pa")
            pb_sb = res.tile([NRF - P, BW], BF, tag="pb")

            nc.vector.memset(ones_sb[:], 1.0)
            make_identity(nc, ident_sb[:])
            # ones column per (chunk, head) folded into v for the softmax sum
            nc.vector.memset(
                v_sb[:].rearrange("p (kc h d) -> p kc h d", h=H, d=DV1)[:, :, :, DV : DV1],
                1.0,
            )
            nc.scalar.dma_start(out=pa_sb[:], in_=pba[:])
            nc.scalar.dma_start(out=pb_sb[:], in_=pbb[:])
            nc.scalar.dma_start(out=wra_sb[:], in_=wrTa[:])
            nc.scalar.dma_start(out=wrb_sb[:], in_=wrTb[:])

            def emit_shift(i):
                """pp matmuls + DRAM round trip for iteration i = (t, h)."""
                t, h = iters[i]
                hh = h // 2
                ho = (h % 2) * DK
                boff = hh * BS + (2 - t) * P
                stg = stgp.tile([P, WB + 1], BF, tag="stg")
                for c4 in range(4):
                    w = min(512, WB - c4 * 512)
                    pp = ppp.tile([P, 512], F32, tag="pp")
                    nc.tensor.matmul(
                        pp[:, :w],
                        qpT_sb[ho : ho + DK, hh * NB + t * P : hh * NB + (t + 1) * P],
                        band_sb[ho : ho + DK, boff + c4 * 512 : boff + c4 * 512 + w],
                        start=True, stop=True,
                    )
                    nc.vector.tensor_copy(stg[:, c4 * 512 : c4 * 512 + w], pp[:, :w])
                slot = dsc.tile([FLAT], BF, tag="slot")
                nc.gpsimd.dma_start(
                    out=slot[:].rearrange("(p w) -> p w", w=WB), in_=stg[:, :WB]
                )
                # shifted readback: flat[127 + p*1662 + j]
                rbt = rbp.tile([P, N], BF, tag="rb")
                nc.scalar.dma_start(
                    out=rbt[:],
                    in_=slot[P - 1 : P - 1 + P * RBS].rearrange(
                        "(p w) -> p w", w=RBS
                    )[:, :N],
                )
                rbts[i] = rbt

            # ---- rel_k band, once per head-pair, 1919-wide window ----
            for hh in range(4):
                for c4 in range(4):
                    w = min(512, BW - c4 * 512)
                    pband = prot.tile([P, 512], F32, tag="ps")
                    nc.tensor.matmul(
                        pband[:, :w],
                        wra_sb[:, hh * P : (hh + 1) * P],
                        pa_sb[:, c4 * 512 : c4 * 512 + w],
                        start=True, stop=False,
                    )
                    nc.tensor.matmul(
                        pband[:, :w],
                        wrb_sb[:, hh * P : (hh + 1) * P],
                        pb_sb[:, c4 * 512 : c4 * 512 + w],
                        start=False, stop=True,
                    )
                    nc.vector.tensor_copy(
                        band_sb[:, hh * BS + c4 * 512 : hh * BS + c4 * 512 + w],
                        pband[:, :w],
                    )

            # ---------------- projections (all bf16) ----------------
            with (
                tc.tile_pool(name="xw", bufs=1) as xw,
                tc.tile_pool(name="psum_mm", bufs=3, space="PSUM") as pkp,
            ):
                xT_sb = xw.tile([P, KC * N], BF, tag="xT")        # 36 KB/part
                xq_sb = xw.tile([P, KC * NB], BF, tag="xq")
                wq_sb = xw.tile([P, KC * 512], BF, tag="wq")
                wk_sb = xw.tile([P, KC * 512], BF, tag="wk")
                wv_sb = xw.tile([P, KC * HDV], BF, tag="wv")      # 36 KB/part

                nc.sync.dma_start(
                    out=xq_sb[:].rearrange("p (kc c) -> p kc c", c=NB),
                    in_=xqT[:].rearrange("(kc p) c -> p kc c", p=P),
                )
                nc.sync.dma_start(
                    out=xT_sb[:].rearrange("p (kc c) -> p kc c", c=N),
                    in_=xT[:].rearrange("(kc p) c -> p kc c", p=P),
                )
                nc.scalar.dma_start(
                    out=wq_sb[:].rearrange("p (kc c) -> p kc c", c=512),
                    in_=wqT[:].rearrange("(kc p) c -> p kc c", p=P),
                )
                nc.scalar.dma_start(out=rcb_sb[:], in_=rcb[:])
                nc.scalar.dma_start(out=rpb_sb[:], in_=rpb[:])
                nc.scalar.dma_start(
                    out=wk_sb[:].rearrange("p (kc c) -> p kc c", c=512),
                    in_=wkT[:].rearrange("(kc p) c -> p kc c", p=P),
                )
                nc.scalar.dma_start(
                    out=wv_sb[:].rearrange("p (kc c) -> p kc c", c=HDV),
                    in_=wvT[:].rearrange("(kc p) c -> p kc c", p=P),
                )
                nc.scalar.dma_start(out=bo_sb[:], in_=bo[:])

                # q projection: 4 M-tiles of 128 rows, own 384 columns
                for m in range(4):
                    pq = pkp.tile([P, NB], F32, tag="pmm")
                    for kc in range(KC):
                        nc.tensor.matmul(
                            pq[:],
                            wq_sb[:, kc * 512 + m * P : kc * 512 + (m + 1) * P],
                            xq_sb[:, kc * NB : (kc + 1) * NB],
                            start=(kc == 0), stop=(kc == KC - 1),
                        )
                    nc.scalar.activation(
                        qcT_sb[:, m * NB : (m + 1) * NB], pq[:], AF.Identity,
                        bias=rcb_sb[:, m : m + 1], scale=scale,
                    )
                    nc.scalar.activation(
                        qpT_sb[:, m * NB : (m + 1) * NB], pq[:], AF.Identity,
                        bias=rpb_sb[:, m : m + 1], scale=scale,
                    )

                # k projection: kT = Wk @ x^T, in 2-Mtile groups (PSUM)
                for g in range(2):
                    for nc3 in range(3):
                        pk = [pkp.tile([P, 512], F32, tag="pmm", name=f"pk{m_}") for m_ in range(2)]
                        for kc in range(KC):
                            for m_ in range(2):
                                m = g * 2 + m_
                                nc.tensor.matmul(
                                    pk[m_][:],
                                    wk_sb[:, kc * 512 + m * P : kc * 512 + (m + 1) * P],
                                    xT_sb[:, kc * N + nc3 * 512 : kc * N + (nc3 + 1) * 512],
                                    start=(kc == 0), stop=(kc == KC - 1),
                                )
                        for m_ in range(2):
                            m = g * 2 + m_
                            nc.vector.tensor_copy(
                                kT_sb[:, m * N + nc3 * 512 : m * N + (nc3 + 1) * 512],
                                pk[m_][:],
                            )

                # v projection, key chunks of 128; shift priming interleaved
                for ms in range(KC):
                    pv3 = [pkp.tile([P, 512], F32, tag="pmm", name=f"pv{n_}") for n_ in range(3)]
                    for kc in range(KC):
                        for n3 in range(3):
                            nc.tensor.matmul(
                                pv3[n3][:],
                                xT_sb[:, kc * N + ms * P : kc * N + (ms + 1) * P],
                                wv_sb[:, kc * HDV + n3 * 512 : kc * HDV + (n3 + 1) * 512],
                                start=(kc == 0), stop=(kc == KC - 1),
                            )
                    # per-head eviction into 193-wide blocks (col 192 is the
                    # preset ones column); heads 2 and 5 straddle PSUM chunks
                    for h in range(H):
                        c0, c1 = h * DV, (h + 1) * DV
                        while c0 < c1:
                            n3 = c0 // 512
                            ce = min(c1, (n3 + 1) * 512)
                            dst = ms * VW + h * DV1 + (c0 - h * DV)
                            nc.vector.tensor_copy(
                                v_sb[:, dst : dst + (ce - c0)],
                                pv3[n3][:, c0 - n3 * 512 : ce - n3 * 512],
                            )
                            c0 = ce
                    if ms in (3, 6, 9):
                        emit_shift(ms // 3 - 1)

            # ---------------- attention + output, per q-tile ----------------
            with (
                tc.tile_pool(name="att_sb", bufs=1) as asb,
                tc.tile_pool(name="es", bufs=2) as esp,
                tc.tile_pool(name="at", bufs=4) as atp,
                tc.tile_pool(name="av", bufs=2) as avp,
                tc.tile_pool(name="avT", bufs=2) as avTp,
                tc.tile_pool(name="ou", bufs=2) as oup,
                tc.tile_pool(name="psum_av", bufs=1, space="PSUM") as pav,
                tc.tile_pool(name="psum_po", bufs=2, space="PSUM") as pop,
            ):
                wo_sb = asb.tile([P, KC * D], BF, tag="wo")       # 18 KB/part
                nc.sync.dma_start(
                    out=wo_sb[:].rearrange("p (cc c) -> p cc c", c=D),
                    in_=woT[:].rearrange("(cc p) c -> p cc c", p=P),
                )

                def emit_consume(i):
                    """content logits + shifted-rel add (PSUM), exp, transpose."""
                    t, h = iters[i]
                    hh = h // 2
                    ho = (h % 2) * DK
                    rbt = rbts.pop(i)
                    es = esp.tile([P, N], BF, tag="es")
                    for n3 in range(3):
                        ps = prot.tile([P, 512], F32, tag="ps")
                        nc.tensor.matmul(
                            ps[:],
                            qcT_sb[ho : ho + DK, hh * NB + t * P : hh * NB + (t + 1) * P],
                            kT_sb[ho : ho + DK, hh * N + n3 * 512 : hh * N + (n3 + 1) * 512],
                            start=True, stop=False,
                        )
                        nc.tensor.matmul(
                            ps[:],
                            ident_sb[:],
                            rbt[:, n3 * 512 : (n3 + 1) * 512],
                            start=False, stop=True,
                        )
                        nc.scalar.activation(
                            es[:, n3 * 512 : (n3 + 1) * 512], ps[:], AF.Exp,
                        )
                    att_all = atp.tile([P, KC * P], BF, tag="at")
                    nc.sync.dma_start_transpose(
                        att_all[:].rearrange("p (kc c) -> p kc c", c=P), es[:]
                    )
                    return att_all

                atts = {}

                def emit_pv(i):
                    """attnv for iteration i (runs LAG2 heads behind consume).

                    The ones column in v makes pv[:, 192] the softmax row sum."""
                    t, h = iters[i]
                    att_all = atts.pop(i)
                    pv = pav.tile([P, DV1], F32, tag="pav")
                    for kc in range(KC):
                        nc.tensor.matmul(
                            pv[:],
                            att_all[:, kc * P : (kc + 1) * P],
                            v_sb[:, kc * VW + h * DV1 : kc * VW + (h + 1) * DV1],
                            start=(kc == 0), stop=(kc == KC - 1),
                        )
                    recip = smp.tile([P, 1], F32, tag="recip")
                    nc.vector.reciprocal(recip[:], pv[:, DV : DV1])
                    nc.scalar.activation(
                        attnvs[t][:, h * DV : (h + 1) * DV], pv[:, :DV], AF.Copy,
                        scale=recip[:],
                    )

                def emit_po(t):
                    """output projection for tile t (after all its heads)."""
                    avT_all = avTp.tile([P, KC * P], BF, tag="avT")
                    nc.sync.dma_start_transpose(
                        avT_all[:].rearrange("p (cc c) -> p cc c", c=P),
                        attnvs.pop(t)[:],
                    )
                    for n3 in range(3):
                        po = pop.tile([P, 512], F32, tag="po")
                        for cc in range(KC):
                            nc.tensor.matmul(
                                po[:],
                                avT_all[:, cc * P : (cc + 1) * P],
                                wo_sb[:, cc * D + n3 * 512 : cc * D + (n3 + 1) * 512],
                                start=(cc == 0), stop=False,
                            )
                        nc.tensor.matmul(
                            po[:],
                            ones_sb[:],
                            bo_sb[:, n3 * 512 : (n3 + 1) * 512],
                            start=False, stop=True,
                        )
                        ot = oup.tile([P, 512], F32, tag="ou")
                        nc.scalar.copy(ot[:], po[:])
                        nc.sync.dma_start(
                            out=out[t * P : (t + 1) * P, n3 * 512 : (n3 + 1) * 512],
                            in_=ot[:],
                        )

                for i, (t, h) in enumerate(iters):
                    if h == 0:
                        attnvs[t] = avp.tile([P, HDV], BF, tag="attnv", name=f"attnv{t}")
                    if i + LAG < len(iters):
                        emit_shift(i + LAG)
                    atts[i] = emit_consume(i)
                    if i >= LAG2:
                        emit_pv(i - LAG2)
                        tl, hl = iters[i - LAG2]
                        if hl == H - 1:
                            emit_po(tl)
                for i in range(len(iters) - LAG2, len(iters)):
                    emit_pv(i)
                emit_po(NT - 1)

    nc.compile()
    return nc


_CACHE = {}


def _get_nc():
    if "nc" not in _CACHE:
        _CACHE["nc"] = _build_nc()
    return _CACHE["nc"]


def kernel(x, Wq, Wk, Wv, Wrel, Wout, b_out, rel_content_bias, rel_pos_bias):
    x = np.asarray(x, np.float32)
    Wq = np.asarray(Wq, np.float32)
    Wk = np.asarray(Wk, np.float32)
    Wv = np.asarray(Wv, np.float32)
    Wrel = np.asarray(Wrel, np.float32)
    Wout = np.asarray(Wout, np.float32)
    b_out = np.asarray(b_out, np.float32)
    rcb = np.asarray(rel_content_bias, np.float32).reshape(H * DK)
    rpb = np.asarray(rel_pos_bias, np.float32).reshape(H * DK)

    positions = _np_positions()  # (3071, 192) f32, input-independent constant
    posT = np.ascontiguousarray(positions.T).astype(BF16)  # (192, 3071)

    wqT = np.ascontiguousarray(Wq.T).astype(BF16)
    wkT = np.ascontiguousarray(Wk.T).astype(BF16)
    wvT = np.ascontiguousarray(Wv.T).astype(BF16)
    woT = np.ascontiguousarray(Wout.T).astype(BF16)
    wrT = np.ascontiguousarray(Wrel.T).astype(BF16)  # (192, 512)
    rcb_in = np.ascontiguousarray(rcb.reshape(4, P).T)  # (128, 4)
    rpb_in = np.ascontiguousarray(rpb.reshape(4, P).T)
    bo_in = b_out.reshape(1, D).astype(BF16)

    in_maps = []
    for core in range(8):
        bi, ci = core // 4, core % 4
        q0 = ci * NB
        xTb = np.ascontiguousarray(x[bi].T).astype(BF16)  # (dim, n) bf16
        s0 = 1152 - q0  # band window start in the 3071 rel positions
        in_maps.append(
            {
                "xT": xTb,
                "xqT": np.ascontiguousarray(xTb[:, q0 : q0 + NB]),
                "wqT": wqT,
                "wkT": wkT,
                "wvT": wvT,
                "woT": woT,
                "wrTa": np.ascontiguousarray(wrT[:P]),
                "wrTb": np.ascontiguousarray(wrT[P:]),
                "pba": np.ascontiguousarray(posT[:P, s0 : s0 + BW]),
                "pbb": np.ascontiguousarray(posT[P:, s0 : s0 + BW]),
                "rcb": rcb_in,
                "rpb": rpb_in,
                "bo": bo_in,
            }
        )

    nc = _get_nc()
    trace = bool(os.environ.get("KERNEL_TRACE"))
    res = run_bass_kernel_spmd(nc, in_maps, list(range(8)), trace=trace)
    _CACHE["last_res"] = res

    out = np.empty((2, N, D), np.float32)
    for core in range(8):
        bi, ci = core // 4, core % 4
        out[bi, ci * NB : (ci + 1) * NB] = res.results[core]["out"]
    return out


# revision 21
# speedup vs baseline: 1.0008x; 1.0008x over previous
"""Trainium2 Bass kernel for Enformer-style relative-position attention.

Problem: b=2, n=1536, dim=1536, 8 heads, dk=64, dv=192, rel-pos features=192.

Sharding: pure sequence sharding, no collectives. 8 cores = 2 batches x 4
query-row slices of 384. Each core computes full k/v for its batch
(duplicated within the 4-core batch group), attention + output projection
for its 384 query rows. Host concatenates the 8 (384, 1536) slices.

All matmuls run in bf16. A single bf16 x^T resident serves the k, q and
v projections. The rel-pos band rel_k^T is computed once per head-pair
over a 1919-wide window covering all three query tiles.

relative_shift is realized as a DRAM round trip: the pre-shift band
S_pre (128 x 1663) is written row-major to a flat DRAM scratch, and read
back with row stride 1662 starting at offset 127, which lands
shifted[p, j] = S_pre[p, 127 - p + j] exactly.

The shifted band is accumulated onto the content logits directly in PSUM
via an identity matmul, and the softmax exp reads PSUM. The attention
stages are emitted in a software-pipelined order (shift runs LAG=3 heads
ahead, attnv runs LAG2=2 heads behind) because the Tile scheduler emits
a static per-engine order.
"""

import math
import os

import ml_dtypes
import numpy as np

import concourse.bass as bass
import concourse.mybir as mybir
import concourse.tile as tile
from concourse import bacc
from concourse.bass_utils import run_bass_kernel_spmd
from concourse.masks import make_identity

BF16 = ml_dtypes.bfloat16
F32 = mybir.dt.float32
BF = mybir.dt.bfloat16
AF = mybir.ActivationFunctionType
AX = mybir.AxisListType
ALU = mybir.AluOpType

P = 128
N = 1536          # sequence length
D = 1536          # model dim
H = 8             # heads
DK = 64           # key dim per head
DV = 192          # value dim per head
HDV = H * DV      # 1536
NB = 384          # query rows per core
NT = NB // P      # q-tiles per core = 3
KC = D // P       # contraction chunks over model dim = 12
WB = N + P - 1    # pre-shift band width = 1663
RBS = WB - 1      # readback row stride = 1662
FLAT = P * WB     # flat scratch elements per (tile, head) = 212864
NRF = 192         # rel-pos feature size
BW = WB + 2 * P   # full band window per head-pair = 1919
BS = BW + 1       # band column stride = 1920

LAG = 3           # shift round-trip emitted this many heads ahead
LAG2 = 3          # attnv emitted this many heads behind
DV1 = DV + 1      # per-head v block incl. ones column for the softmax sum
VW = H * DV1      # v chunk width = 1544


def _np_positions():
    """numpy replication of reference.get_positional_embed(1536, 192)."""
    from scipy.special import gammaln as sp_gammaln

    n, feature_size = N, NRF
    dist = np.arange(-n + 1, n)
    adist = np.abs(dist).astype(np.float64)[:, None]
    num_basis = feature_size // 6
    max_range = math.log(n) / math.log(2.0)
    half_life = 2.0 ** np.linspace(3.0, max_range, num_basis)
    feat_exp = np.exp(-math.log(2.0) / half_life[None, :] * adist)
    center_widths = 2.0 ** np.arange(1, num_basis + 1) - 1.0
    feat_cm = (center_widths[None, :] > adist).astype(np.float64)
    stddev = n / (2 * num_basis)
    start_mean = n / num_basis
    mean = np.linspace(start_mean, float(n), num_basis)[None, :]
    concentration = (mean / stddev) ** 2
    rate = mean / (stddev**2)
    with np.errstate(divide="ignore", invalid="ignore"):
        xl = np.where(
            adist > 0,
            (concentration - 1.0) * np.log(np.where(adist > 0, adist, 1.0)),
            0.0,
        )
        xl = np.where((adist == 0) & (concentration - 1.0 != 0), -np.inf, xl)
    log_unnorm = xl - rate * adist
    log_norm = sp_gammaln(concentration) - concentration * np.log(rate)
    prob = np.exp(log_unnorm - log_norm) + 1e-8
    feat_gamma = prob / np.max(prob, axis=-1, keepdims=True)
    emb = np.concatenate([feat_exp, feat_cm, feat_gamma], axis=-1)
    sign = np.sign(dist).astype(np.float64)[:, None]
    return np.concatenate([emb, sign * emb], axis=-1).astype(np.float32)


def _build_nc():
    nc = bacc.Bacc("TRN2", target_bir_lowering=False)

    xT = nc.dram_tensor("xT", [D, N], BF, kind="ExternalInput")
    xqT = nc.dram_tensor("xqT", [D, NB], BF, kind="ExternalInput")
    wqT = nc.dram_tensor("wqT", [D, H * DK], BF, kind="ExternalInput")
    wkT = nc.dram_tensor("wkT", [D, H * DK], BF, kind="ExternalInput")
    wvT = nc.dram_tensor("wvT", [D, HDV], BF, kind="ExternalInput")
    woT = nc.dram_tensor("woT", [HDV, D], BF, kind="ExternalInput")
    wrTa = nc.dram_tensor("wrTa", [P, H * DK], BF, kind="ExternalInput")
    wrTb = nc.dram_tensor("wrTb", [NRF - P, H * DK], BF, kind="ExternalInput")
    # positions^T band window (covers all 3 local q-tiles), feature-split
    pba = nc.dram_tensor("pba", [P, BW], BF, kind="ExternalInput")
    pbb = nc.dram_tensor("pbb", [NRF - P, BW], BF, kind="ExternalInput")
    rcb = nc.dram_tensor("rcb", [P, 4], F32, kind="ExternalInput")
    rpb = nc.dram_tensor("rpb", [P, 4], F32, kind="ExternalInput")
    bo = nc.dram_tensor("bo", [1, D], BF, kind="ExternalInput")

    out = nc.dram_tensor("out", [NB, D], F32, kind="ExternalOutput")

    scale = DK ** -0.5
    iters = [(t, h) for t in range(NT) for h in range(H)]
    rbts = {}     # iter idx -> readback tile
    attnvs = {}   # tile idx -> attnv tile

    with tile.TileContext(nc) as tc:
        with (
            tc.tile_pool(name="res", bufs=1) as res,
            tc.tile_pool(name="psum_mm", bufs=5, space="PSUM") as pmm,
            tc.tile_pool(name="psum_pp", bufs=2, space="PSUM") as ppp,
            tc.tile_pool(name="stg", bufs=2) as stgp,
            tc.tile_pool(name="rb", bufs=4) as rbp,
            tc.tile_pool(name="sm", bufs=8) as smp,
            tc.tile_pool(name="dram", bufs=8, space="DRAM") as dsc,
        ):
            # long-lived SBUF residents
            kT_sb = res.tile([P, 4 * N], BF, tag="kT")            # 12 KB/part
            v_sb = res.tile([P, KC * VW], BF, tag="v")            # 37 KB/part
            qcT_sb = res.tile([P, 4 * NB], BF, tag="qcT")
            qpT_sb = res.tile([P, 4 * NB], BF, tag="qpT")
            band_sb = res.tile([P, 4 * BS], BF, tag="band")       # 15.4 KB/part
            rcb_sb = res.tile([P, 4], F32, tag="rcb")
            rpb_sb = res.tile([P, 4], F32, tag="rpb")
            bo_sb = res.tile([1, D], BF, tag="bo")
            ones_sb = res.tile([1, P], BF, tag="ones")
            ident_sb = res.tile([P, P], BF, tag="ident")
            wra_sb = res.tile([P, H * DK], BF, tag="wra")
            wrb_sb = res.tile([NRF - P, H * DK], BF, tag="wrb")
            pa_sb = res.tile([P, BW], BF, tag="pa")
            pb_sb = res.tile([NRF - P, BW], BF, tag="pb")

            nc.vector.memset(ones_sb[:], 1.0)
            make_identity(nc, ident_sb[:])
            # ones column per (chunk, head) folded into v for the softmax sum
            nc.vector.memset(
                v_sb[:].rearrange("p (kc h d) -> p kc h d", h=H, d=DV1)[:, :, :, DV : DV1],
                1.0,
            )
            nc.scalar.dma_start(out=pa_sb[:], in_=pba[:])
            nc.scalar.dma_start(out=pb_sb[:], in_=pbb[:])
            nc.scalar.dma_start(out=wra_sb[:], in_=wrTa[:])
            nc.scalar.dma_start(out=wrb_sb[:], in_=wrTb[:])

            def emit_shift(i):
                """pp matmuls + DRAM round trip for iteration i = (t, h)."""
                t, h = iters[i]
                hh = h // 2
                ho = (h % 2) * DK
                boff = hh * BS + (2 - t) * P
                stg = stgp.tile([P, WB + 1], BF, tag="stg")
                for c4 in range(4):
                    w = min(512, WB - c4 * 512)
                    pp = ppp.tile([P, 512], F32, tag="pp")
                    nc.tensor.matmul(
                        pp[:, :w],
                        qpT_sb[ho : ho + DK, hh * NB + t * P : hh * NB + (t + 1) * P],
                        band_sb[ho : ho + DK, boff + c4 * 512 : boff + c4 * 512 + w],
                        start=True, stop=True,
                    )
                    eng = nc.vector if c4 < 2 else nc.scalar
                    if c4 < 2:
                        nc.vector.tensor_copy(stg[:, c4 * 512 : c4 * 512 + w], pp[:, :w])
                    else:
                        nc.scalar.copy(stg[:, c4 * 512 : c4 * 512 + w], pp[:, :w])
                slot = dsc.tile([FLAT], BF, tag="slot")
                nc.gpsimd.dma_start(
                    out=slot[:].rearrange("(p w) -> p w", w=WB), in_=stg[:, :WB]
                )
                # shifted readback: flat[127 + p*1662 + j]
                rbt = rbp.tile([P, N], BF, tag="rb")
                nc.scalar.dma_start(
                    out=rbt[:],
                    in_=slot[P - 1 : P - 1 + P * RBS].rearrange(
                        "(p w) -> p w", w=RBS
                    )[:, :N],
                )
                rbts[i] = rbt

            # ---- rel_k band, once per head-pair, 1919-wide window ----
            for hh in range(4):
                for c4 in range(4):
                    w = min(512, BW - c4 * 512)
                    pband = pmm.tile([P, 512], F32, tag="mm")
                    nc.tensor.matmul(
                        pband[:, :w],
                        wra_sb[:, hh * P : (hh + 1) * P],
                        pa_sb[:, c4 * 512 : c4 * 512 + w],
                        start=True, stop=False,
                    )
                    nc.tensor.matmul(
                        pband[:, :w],
                        wrb_sb[:, hh * P : (hh + 1) * P],
                        pb_sb[:, c4 * 512 : c4 * 512 + w],
                        start=False, stop=True,
                    )
                    nc.vector.tensor_copy(
                        band_sb[:, hh * BS + c4 * 512 : hh * BS + c4 * 512 + w],
                        pband[:, :w],
                    )

            # ---------------- projections (all bf16) ----------------
            with tc.tile_pool(name="xw", bufs=1) as xw:
                xT_sb = xw.tile([P, KC * N], BF, tag="xT")        # 36 KB/part
                xq_sb = xw.tile([P, KC * NB], BF, tag="xq")
                wq_sb = xw.tile([P, KC * 512], BF, tag="wq")
                wk_sb = xw.tile([P, KC * 512], BF, tag="wk")
                wv_sb = xw.tile([P, KC * HDV], BF, tag="wv")      # 36 KB/part

                nc.sync.dma_start(
                    out=xq_sb[:].rearrange("p (kc c) -> p kc c", c=NB),
                    in_=xqT[:].rearrange("(kc p) c -> p kc c", p=P),
                )
                nc.sync.dma_start(
                    out=xT_sb[:].rearrange("p (kc c) -> p kc c", c=N),
                    in_=xT[:].rearrange("(kc p) c -> p kc c", p=P),
                )
                nc.scalar.dma_start(
                    out=wq_sb[:].rearrange("p (kc c) -> p kc c", c=512),
                    in_=wqT[:].rearrange("(kc p) c -> p kc c", p=P),
                )
                nc.scalar.dma_start(out=rcb_sb[:], in_=rcb[:])
                nc.scalar.dma_start(out=rpb_sb[:], in_=rpb[:])
                nc.scalar.dma_start(
                    out=wk_sb[:].rearrange("p (kc c) -> p kc c", c=512),
                    in_=wkT[:].rearrange("(kc p) c -> p kc c", p=P),
                )
                nc.scalar.dma_start(
                    out=wv_sb[:].rearrange("p (kc c) -> p kc c", c=HDV),
                    in_=wvT[:].rearrange("(kc p) c -> p kc c", p=P),
                )
                nc.scalar.dma_start(out=bo_sb[:], in_=bo[:])

                # q projection: 4 M-tiles of 128 rows, own 384 columns
                for m in range(4):
                    pq = pmm.tile([P, NB], F32, tag="mm")
                    for kc in range(KC):
                        nc.tensor.matmul(
                            pq[:],
                            wq_sb[:, kc * 512 + m * P : kc * 512 + (m + 1) * P],
                            xq_sb[:, kc * NB : (kc + 1) * NB],
                            start=(kc == 0), stop=(kc == KC - 1),
                        )
                    nc.scalar.activation(
                        qcT_sb[:, m * NB : (m + 1) * NB], pq[:], AF.Identity,
                        bias=rcb_sb[:, m : m + 1], scale=scale,
                    )
                    nc.scalar.activation(
                        qpT_sb[:, m * NB : (m + 1) * NB], pq[:], AF.Identity,
                        bias=rpb_sb[:, m : m + 1], scale=scale,
                    )

                # k projection: kT = Wk @ x^T, in 2-Mtile groups (PSUM)
                for g in range(2):
                    for nc3 in range(3):
                        pk = [pmm.tile([P, 512], F32, tag="mm", name=f"pk{m_}") for m_ in range(2)]
                        for kc in range(KC):
                            for m_ in range(2):
                                m = g * 2 + m_
                                nc.tensor.matmul(
                                    pk[m_][:],
                                    wk_sb[:, kc * 512 + m * P : kc * 512 + (m + 1) * P],
                                    xT_sb[:, kc * N + nc3 * 512 : kc * N + (nc3 + 1) * 512],
                                    start=(kc == 0), stop=(kc == KC - 1),
                                )
                        for m_ in range(2):
                            m = g * 2 + m_
                            nc.vector.tensor_copy(
                                kT_sb[:, m * N + nc3 * 512 : m * N + (nc3 + 1) * 512],
                                pk[m_][:],
                            )

                # v projection, key chunks of 128; shift priming interleaved
                for ms in range(KC):
                    pv3 = [pmm.tile([P, 512], F32, tag="mm", name=f"pv{n_}") for n_ in range(3)]
                    for kc in range(KC):
                        for n3 in range(3):
                            nc.tensor.matmul(
                                pv3[n3][:],
                                xT_sb[:, kc * N + ms * P : kc * N + (ms + 1) * P],
                                wv_sb[:, kc * HDV + n3 * 512 : kc * HDV + (n3 + 1) * 512],
                                start=(kc == 0), stop=(kc == KC - 1),
                            )
                    # per-head eviction into 193-wide blocks (col 192 is the
                    # preset ones column); heads 2 and 5 straddle PSUM chunks
                    for h in range(H):
                        c0, c1 = h * DV, (h + 1) * DV
                        while c0 < c1:
                            n3 = c0 // 512
                            ce = min(c1, (n3 + 1) * 512)
                            dst = ms * VW + h * DV1 + (c0 - h * DV)
                            nc.vector.tensor_copy(
                                v_sb[:, dst : dst + (ce - c0)],
                                pv3[n3][:, c0 - n3 * 512 : ce - n3 * 512],
                            )
                            c0 = ce
                    if ms in (3, 6, 9):
                        emit_shift(ms // 3 - 1)

            # ---------------- attention + output, per q-tile ----------------
            with (
                tc.tile_pool(name="att_sb", bufs=1) as asb,
                tc.tile_pool(name="es", bufs=2) as esp,
                tc.tile_pool(name="at", bufs=4) as atp,
                tc.tile_pool(name="av", bufs=2) as avp,
                tc.tile_pool(name="avT", bufs=2) as avTp,
                tc.tile_pool(name="ou", bufs=2) as oup,
                tc.tile_pool(name="psum_av", bufs=1, space="PSUM") as pav,
            ):
                wo_sb = asb.tile([P, KC * D], BF, tag="wo")       # 18 KB/part
                nc.sync.dma_start(
                    out=wo_sb[:].rearrange("p (cc c) -> p cc c", c=D),
                    in_=woT[:].rearrange("(cc p) c -> p cc c", p=P),
                )

                def emit_consume(i):
                    """content logits + shifted-rel add (PSUM), exp, transpose."""
                    t, h = iters[i]
                    hh = h // 2
                    ho = (h % 2) * DK
                    rbt = rbts.pop(i)
                    es = esp.tile([P, N], BF, tag="es")
                    for n3 in range(3):
                        ps = pmm.tile([P, 512], F32, tag="mm")
                        nc.tensor.matmul(
                            ps[:],
                            qcT_sb[ho : ho + DK, hh * NB + t * P : hh * NB + (t + 1) * P],
                            kT_sb[ho : ho + DK, hh * N + n3 * 512 : hh * N + (n3 + 1) * 512],
                            start=True, stop=False,
                        )
                        nc.tensor.matmul(
                            ps[:],
                            ident_sb[:],
                            rbt[:, n3 * 512 : (n3 + 1) * 512],
                            start=False, stop=True,
                        )
                        nc.scalar.activation(
                            es[:, n3 * 512 : (n3 + 1) * 512], ps[:], AF.Exp,
                        )
                    att_all = atp.tile([P, KC * P], BF, tag="at")
                    nc.sync.dma_start_transpose(
                        att_all[:].rearrange("p (kc c) -> p kc c", c=P), es[:]
                    )
                    return att_all

                atts = {}

                def emit_pv(i):
                    """attnv for iteration i (runs LAG2 heads behind consume).

                    The ones column in v makes pv[:, 192] the softmax row sum."""
                    t, h = iters[i]
                    att_all = atts.pop(i)
                    pv = pav.tile([P, DV1], F32, tag="pav")
                    for kc in range(KC):
                        nc.tensor.matmul(
                            pv[:],
                            att_all[:, kc * P : (kc + 1) * P],
                            v_sb[:, kc * VW + h * DV1 : kc * VW + (h + 1) * DV1],
                            start=(kc == 0), stop=(kc == KC - 1),
                        )
                    recip = smp.tile([P, 1], F32, tag="recip")
                    nc.vector.reciprocal(recip[:], pv[:, DV : DV1])
                    nc.scalar.activation(
                        attnvs[t][:, h * DV : (h + 1) * DV], pv[:, :DV], AF.Copy,
                        scale=recip[:],
                    )

                def emit_po(t):
                    """output projection for tile t (after all its heads)."""
                    avT_all = avTp.tile([P, KC * P], BF, tag="avT")
                    nc.sync.dma_start_transpose(
                        avT_all[:].rearrange("p (cc c) -> p cc c", c=P),
                        attnvs.pop(t)[:],
                    )
                    for n3 in range(3):
                        po = pmm.tile([P, 512], F32, tag="mm")
                        for cc in range(KC):
                            nc.tensor.matmul(
                                po[:],
                                avT_all[:, cc * P : (cc + 1) * P],
                                wo_sb[:, cc * D + n3 * 512 : cc * D + (n3 + 1) * 512],
                                start=(cc == 0), stop=False,
                            )
                        nc.tensor.matmul(
                            po[:],
                            ones_sb[:],
                            bo_sb[:, n3 * 512 : (n3 + 1) * 512],
                            start=False, stop=True,
                        )
                        ot = oup.tile([P, 512], F32, tag="ou")
                        nc.scalar.copy(ot[:], po[:])
                        nc.sync.dma_start(
                            out=out[t * P : (t + 1) * P, n3 * 512 : (n3 + 1) * 512],
                            in_=ot[:],
                        )

                for i, (t, h) in enumerate(iters):
                    if h == 0:
                        attnvs[t] = avp.tile([P, HDV], BF, tag="attnv", name=f"attnv{t}")
                    if i + LAG < len(iters):
                        emit_shift(i + LAG)
                    atts[i] = emit_consume(i)
                    if i >= LAG2:
                        emit_pv(i - LAG2)
                        tl, hl = iters[i - LAG2]
                        if hl == H - 1:
                            emit_po(tl)
                for i in range(len(iters) - LAG2, len(iters)):
                    emit_pv(i)
                emit_po(NT - 1)

    nc.compile()
    return nc


_CACHE = {}


def _get_nc():
    if "nc" not in _CACHE:
        _CACHE["nc"] = _build_nc()
    return _CACHE["nc"]


def kernel(x, Wq, Wk, Wv, Wrel, Wout, b_out, rel_content_bias, rel_pos_bias):
    x = np.asarray(x, np.float32)
    Wq = np.asarray(Wq, np.float32)
    Wk = np.asarray(Wk, np.float32)
    Wv = np.asarray(Wv, np.float32)
    Wrel = np.asarray(Wrel, np.float32)
    Wout = np.asarray(Wout, np.float32)
    b_out = np.asarray(b_out, np.float32)
    rcb = np.asarray(rel_content_bias, np.float32).reshape(H * DK)
    rpb = np.asarray(rel_pos_bias, np.float32).reshape(H * DK)

    positions = _np_positions()  # (3071, 192) f32, input-independent constant
    posT = np.ascontiguousarray(positions.T).astype(BF16)  # (192, 3071)

    wqT = np.ascontiguousarray(Wq.T).astype(BF16)
    wkT = np.ascontiguousarray(Wk.T).astype(BF16)
    wvT = np.ascontiguousarray(Wv.T).astype(BF16)
    woT = np.ascontiguousarray(Wout.T).astype(BF16)
    wrT = np.ascontiguousarray(Wrel.T).astype(BF16)  # (192, 512)
    rcb_in = np.ascontiguousarray(rcb.reshape(4, P).T)  # (128, 4)
    rpb_in = np.ascontiguousarray(rpb.reshape(4, P).T)
    bo_in = b_out.reshape(1, D).astype(BF16)

    in_maps = []
    for core in range(8):
        bi, ci = core // 4, core % 4
        q0 = ci * NB
        xTb = np.ascontiguousarray(x[bi].T).astype(BF16)  # (dim, n) bf16
        s0 = 1152 - q0  # band window start in the 3071 rel positions
        in_maps.append(
            {
                "xT": xTb,
                "xqT": np.ascontiguousarray(xTb[:, q0 : q0 + NB]),
                "wqT": wqT,
                "wkT": wkT,
                "wvT": wvT,
                "woT": woT,
                "wrTa": np.ascontiguousarray(wrT[:P]),
                "wrTb": np.ascontiguousarray(wrT[P:]),
                "pba": np.ascontiguousarray(posT[:P, s0 : s0 + BW]),
                "pbb": np.ascontiguousarray(posT[P:, s0 : s0 + BW]),
                "rcb": rcb_in,
                "rpb": rpb_in,
                "bo": bo_in,
            }
        )

    nc = _get_nc()
    trace = bool(os.environ.get("KERNEL_TRACE"))
    res = run_bass_kernel_spmd(nc, in_maps, list(range(8)), trace=trace)
    _CACHE["last_res"] = res

    out = np.empty((2, N, D), np.float32)
    for core in range(8):
        bi, ci = core // 4, core % 4
        out[bi, ci * NB : (ci + 1) * NB] = res.results[core]["out"]
    return out
